# revision 14
# baseline (speedup 1.0000x reference)
"""Trainium2 Bass kernel for nn_DyGraphAtt2d (kNN graph + GAT), batch-parallel on 8 cores.

Per core (one batch element):
  x_c [C=128, N=4096] fp32
  scores S[n,m] = dot(x_n,x_m) - 0.5*||x_m||^2  (same per-row order as -dist)
    PE: fp32 Gram matmul + K=3 bf16 matmul folding -sq/2 (3-term bf16 split, ~1e-5 abs exact)
  top-16 per row: DVE max8 / max_index / match_replace on exact fp32 scores
  payload gather: dma_gather of [h(512) | a_src(4) | pad] fp16 rows from DRAM
  GAT softmax over 16 neighbors; weighted sum via DVE per-(g,h) scale + PE identity-matmul
  head mean + bias -> out [N, F] fp32 (host transposes to [B, F, N, 1])
"""
import numpy as np
import ml_dtypes

B, C, N, H, F, K = 8, 128, 4096, 4, 128, 16
NEG_SLOPE = 0.2
PAY = 640           # payload row: [h 512 | a_src 4 | pad 124] fp16 = 1280 B (%256==0)
NBLK = N // 128

_CACHE = {}
PROFILE = False      # set True by test harness to request an NTFF trace
LAST_RESULT = None   # BassKernelResults of the most recent kernel() call


def emit(tc, nblocks, x_d, wcat_d, ones3_d, ident_d, biasrep_d, out_d, pay_d):
    from contextlib import ExitStack
    import concourse.bass as bass
    import concourse.mybir as mybir

    dt = mybir.dt
    f32, f16, bf16, i16, u16 = dt.float32, dt.float16, dt.bfloat16, dt.int16, dt.uint16
    AX = mybir.AxisListType
    OP = mybir.AluOpType
    AF = mybir.ActivationFunctionType
    nc = tc.nc

    with ExitStack() as ctx:
        consts = ctx.enter_context(tc.tile_pool(name="consts", bufs=1))
        spool = ctx.enter_context(tc.tile_pool(name="spool", bufs=2))
        paypool = ctx.enter_context(tc.tile_pool(name="paypool", bufs=3))
        gatpool = ctx.enter_context(tc.tile_pool(name="gatpool", bufs=2))
        scrpool = ctx.enter_context(tc.tile_pool(name="scrpool", bufs=3))
        small = ctx.enter_context(tc.tile_pool(name="small", bufs=4))
        sqtmp = ctx.enter_context(tc.tile_pool(name="sqtmp", bufs=1))
        ps_mm = ctx.enter_context(tc.tile_pool(name="ps_mm", bufs=3, space="PSUM"))
        ps_acc = ctx.enter_context(tc.tile_pool(name="ps_acc", bufs=2, space="PSUM"))

        # ---- resident inputs ----
        x_sb = consts.tile([C, N], f32)
        nc.sync.dma_start(out=x_sb, in_=x_d)
        wcat_sb = consts.tile([C, 521], f32)
        nc.sync.dma_start(out=wcat_sb, in_=wcat_d)
        ones3_sb = consts.tile([3, 128], bf16)
        nc.sync.dma_start(out=ones3_sb, in_=ones3_d)
        ident_sb = consts.tile([128, 128], f16)
        nc.sync.dma_start(out=ident_sb, in_=ident_d)
        biasrep_sb = consts.tile([128, 128], f32)
        nc.sync.dma_start(out=biasrep_sb, in_=biasrep_d)

        adst_all = consts.tile([128, NBLK * H], f32)
        nbr_all = consts.tile([128, nblocks * K], dt.uint32)
        s3t = consts.tile([3, N], bf16)     # bf16 3-term split of (-sq/2)
        sq_f32 = consts.tile([1, N], f32)

        # ---- sq[m] = sum_c x^2 via ones-matmul partition reduce ----
        xsq = spool.tile([C, N], f32, tag="s")
        nc.scalar.square(xsq, x_sb)
        for j in range(N // 512):
            ps = ps_mm.tile([1, 512], f32, tag="mm")
            nc.tensor.matmul(ps, lhsT=wcat_sb[:, 520:521],
                             rhs=xsq[:, j * 512:(j + 1) * 512], start=True, stop=True)
            nc.scalar.copy(sq_f32[:, j * 512:(j + 1) * 512], ps)
        r1 = sqtmp.tile([1, N], f32, tag="sq1")
        sf = sqtmp.tile([1, N], f32, tag="sq2")
        sb0 = sqtmp.tile([1, N], bf16, tag="sb0")
        sb1 = sqtmp.tile([1, N], bf16, tag="sb1")
        sb2 = sqtmp.tile([1, N], bf16, tag="sb2")
        nc.vector.tensor_scalar_mul(sq_f32, sq_f32, -0.5)   # sq_f32 := -sq/2
        nc.vector.tensor_copy(sb0, sq_f32)
        nc.vector.tensor_copy(sf, sb0)
        nc.vector.tensor_sub(r1, sq_f32, sf)
        nc.vector.tensor_copy(sb1, r1)
        nc.vector.tensor_copy(sf, sb1)
        nc.vector.tensor_sub(r1, r1, sf)
        nc.vector.tensor_copy(sb2, r1)
        nc.sync.dma_start(out=s3t[0:1, :], in_=sb0)
        nc.sync.dma_start(out=s3t[1:2, :], in_=sb1)
        nc.sync.dma_start(out=s3t[2:3, :], in_=sb2)

        # ---- h / a_src / a_dst + payload rows to DRAM (always all nodes) ----
        for nt in range(NBLK):
            lhs = x_sb[:, nt * 128:(nt + 1) * 128]
            ph = ps_mm.tile([128, 512], f32, tag="mm")
            nc.tensor.matmul(ph, lhsT=lhs, rhs=wcat_sb[:, 0:512], start=True, stop=True)
            pa = ps_mm.tile([128, 8], f32, tag="mm")
            nc.tensor.matmul(pa, lhsT=lhs, rhs=wcat_sb[:, 512:520], start=True, stop=True)
            pay = paypool.tile([128, PAY], f16, tag="pay")
            nc.scalar.copy(pay[:, 0:512], ph)
            nc.scalar.copy(pay[:, 512:516], pa[:, 0:4])
            nc.vector.memset(pay[:, 516:PAY], 0.0)
            nc.scalar.copy(adst_all[:, nt * H:(nt + 1) * H], pa[:, 4:8])
            nc.sync.dma_start(out=pay_d[nt * 128:(nt + 1) * 128, :], in_=pay)

        # ---- scores + exact fp32 top-16 selection ----
        import concourse.bass as bass_mod
        for bk in range(nblocks):
            S = spool.tile([128, N], f32, tag="s")
            lhs = x_sb[:, bk * 128:(bk + 1) * 128]
            for j in range(N // 512):
                ps = ps_mm.tile([128, 512], f32, tag="mm")
                nc.tensor.matmul(ps, lhsT=lhs, rhs=x_sb[:, j * 512:(j + 1) * 512],
                                 start=True, stop=False)
                nc.tensor.matmul(ps, lhsT=ones3_sb, rhs=s3t[:, j * 512:(j + 1) * 512],
                                 start=False, stop=True)
                nc.scalar.copy(S[:, j * 512:(j + 1) * 512], ps)
            v1 = small.tile([128, 8], f32, tag="v1")
            v2 = small.tile([128, 8], f32, tag="v2")
            nc.vector.max(out=v1, in_=S)
            nc.vector.max_index(out=nbr_all[:, bk * K:bk * K + 8], in_max=v1, in_values=S)
            nc.vector.match_replace(out=S, in_to_replace=v1, in_values=S,
                                    imm_value=-3.0e38)
            nc.vector.max(out=v2, in_=S)
            nc.vector.max_index(out=nbr_all[:, bk * K + 8:bk * K + 16], in_max=v2,
                                in_values=S)

        # ---- gather + softmax + weighted sum + output ----
        for bk in range(nblocks):
            gat = gatpool.tile([128, K, PAY], f16, tag="gat")
            for g in range(K):
                nc.gpsimd.indirect_dma_start(
                    out=gat[:, g, :], out_offset=None, in_=pay_d,
                    in_offset=bass_mod.IndirectOffsetOnAxis(
                        ap=nbr_all[:, bk * K + g:bk * K + g + 1], axis=0))

            asrc = small.tile([128, H, K], f32, tag="asrc")
            nc.scalar.copy(asrc.rearrange("p h k -> p k h"), gat[:, :, 512:516])
            e = small.tile([128, H, K], f32, tag="e")
            nc.vector.tensor_add(
                e, asrc, adst_all[:, bk * H:(bk + 1) * H].to_broadcast([128, H, K]))
            t1 = small.tile([128, H, K], f32, tag="t1")
            nc.vector.tensor_scalar_mul(t1, e, NEG_SLOPE)
            nc.vector.tensor_max(e, e, t1)
            m4 = small.tile([128, H], f32, tag="m4")
            nc.vector.reduce_max(m4, e, axis=AX.X)
            nc.vector.tensor_sub(e, e, m4.to_broadcast([128, H, K]))
            ex = small.tile([128, H, K], f32, tag="ex")
            nc.scalar.activation(ex, e, AF.Exp)
            z4 = small.tile([128, H], f32, tag="z4")
            nc.vector.tensor_reduce(z4, ex, axis=AX.X, op=OP.add)
            rz = small.tile([128, H], f32, tag="rz")
            nc.vector.reciprocal(rz, z4)
            nc.vector.tensor_scalar_mul(rz, rz, 1.0 / H)
            w = small.tile([128, H, K], f32, tag="w")
            nc.vector.tensor_mul(w, ex, rz.to_broadcast([128, H, K]))

            po = ps_acc.tile([128, 512], f32, tag="acc")
            for g in range(K):
                scr = scrpool.tile([128, 512], f16, tag="scr")
                for h in range(H):
                    nc.vector.tensor_scalar_mul(
                        scr[:, h * 128:(h + 1) * 128],
                        gat[:, g, h * 128:(h + 1) * 128],
                        w[:, h, g:g + 1])
                nc.tensor.matmul(po, lhsT=ident_sb, rhs=scr,
                                 start=(g == 0), stop=(g == K - 1))
            t = small.tile([128, 128], f32, tag="hm0")
            nc.vector.tensor_add(t, po[:, 0:128], biasrep_sb)
            nc.vector.tensor_add(t, t, po[:, 128:256])
            nc.vector.tensor_add(t, t, po[:, 256:384])
            o_sb = small.tile([128, 128], f32, tag="osb")
            nc.vector.tensor_add(o_sb, t, po[:, 384:512])
            nc.sync.dma_start(out=out_d[bk * 128:(bk + 1) * 128, :], in_=o_sb)


def host_consts(W, att_src, att_dst, bias):
    """Host-precomputed constant inputs shared by all cores."""
    W = np.asarray(W, np.float32)
    Wa_src = np.einsum("chf,hf->ch", W.reshape(C, H, F).astype(np.float64),
                       np.asarray(att_src, np.float64)).astype(np.float32)
    Wa_dst = np.einsum("chf,hf->ch", W.reshape(C, H, F).astype(np.float64),
                       np.asarray(att_dst, np.float64)).astype(np.float32)
    wcat = np.concatenate([W, Wa_src, Wa_dst, np.ones((C, 1), np.float32)],
                          axis=1)                                   # [128, 521]
    ones3 = np.ones((3, 128), ml_dtypes.bfloat16)
    ident = np.eye(128, dtype=np.float16)
    biasrep = np.broadcast_to(np.asarray(bias, np.float32)[None, :],
                              (128, F)).copy()
    return wcat, ones3, ident, biasrep


def build_program(nblocks=NBLK):
    import concourse.bacc as bacc
    import concourse.tile as tile
    import concourse.mybir as mybir
    dt = mybir.dt

    nc = bacc.Bacc("TRN2", target_bir_lowering=False)
    x_d = nc.dram_tensor("x0", [C, N], dt.float32, kind="ExternalInput").ap()
    wcat_d = nc.dram_tensor("wcat0", [C, 521], dt.float32, kind="ExternalInput").ap()
    ones3_d = nc.dram_tensor("ones30", [3, 128], dt.bfloat16, kind="ExternalInput").ap()
    ident_d = nc.dram_tensor("ident0", [128, 128], dt.float16, kind="ExternalInput").ap()
    biasrep_d = nc.dram_tensor("biasrep0", [128, 128], dt.float32,
                               kind="ExternalInput").ap()
    out_d = nc.dram_tensor("out0", [N, F], dt.float32, kind="ExternalOutput").ap()
    pay_d = nc.dram_tensor("pay", [N, PAY], dt.float16, kind="Internal").ap()

    with tile.TileContext(nc) as tc:
        emit(tc, nblocks, x_d, wcat_d, ones3_d, ident_d, biasrep_d, out_d, pay_d)
    nc.compile()
    return nc


def kernel(x, W, att_src, att_dst, bias, k):
    assert int(k) == K
    from concourse.bass_utils import run_bass_kernel_spmd

    if "nc" not in _CACHE:
        _CACHE["nc"] = build_program()
    nc = _CACHE["nc"]

    x = np.asarray(x, np.float32)
    wcat, ones3, ident, biasrep = host_consts(W, att_src, att_dst, bias)
    in_maps = []
    for b in range(B):
        in_maps.append({
            "x0": np.ascontiguousarray(x[b, :, :, 0]),
            "wcat0": wcat, "ones30": ones3, "ident0": ident, "biasrep0": biasrep,
        })
    global LAST_RESULT
    res = run_bass_kernel_spmd(nc, in_maps, core_ids=list(range(B)), trace=PROFILE)
    LAST_RESULT = res
    out = np.empty((B, F, N, 1), np.float32)
    for b in range(B):
        out[b, :, :, 0] = res.results[b]["out0"].T
    return out
